# revision 2
# baseline (speedup 1.0000x reference)
"""Trainium2 Bass kernel for nn_CayleyFilter (gnn_message_passing).

Math: the reference's Jacobi step degenerates — its SpMM terms cancel
algebraically:
    tr = (offr + diag*zr) + zi - offr == diag*zr + zi   (+- fp rounding noise)
    ti = (offi + diag*zi) - zr - offi == diag*zi - zr
so each Cayley iteration is an elementwise multiply by the per-node
unit-modulus complex scalar s_p = (d_p - i)^2 / (d_p^2 + 1).  Hence
    z_k = s^k (x)   and the whole module collapses to one GEMM:
    out[(n,p), o] = sum_{g,c} coef_g[p] * x[n,c,p] * W2[(g,c), o]
with coef_g in {Re(s^k), Im(s^k)} (18 groups) and W2 = 2*[Wr; Wi].

Device kernel (per core, data-parallel over batch N, N_local = 4):
  - F^T chunks (128 contract-rows=(g,c), 512 p) = coefR (c-replicated coef)
    * xrep (x replicated x4 over partition groups), built CONCURRENTLY on
    VectorE and GpSimdE: ScalarE stages the x tile into PSUM so VectorE's
    tensor_tensor reads PSUM + its dedicated SBUF port, leaving the shared
    DVE/GpSimd SBUF port pair free for GpSimdE's tensor_tensor.
  - TensorE: psum(64 o, 512 p) += Wchunk(128, 64).T @ F^T chunk, 5 chunks
    (640-row zero-padded contraction), float32r operands (full-rate fp32).
  - out written as (n, C_out, p); host transposes/reshapes.
"""

import numpy as np

N, C, M, MSIDE, COUT, ORDER = 32, 32, 4096, 64, 64, 8
NCORES = 8
NLOC = N // NCORES            # 4 batches per core
KTOT = ORDER + 1              # 9
NGRP = 2 * KTOT               # 18 coefficient groups (real k, imag k)
NCHUNK = 5                    # contraction chunks of 128 rows (640 padded)
PT = 512                      # p tile (matmul moving free dim, fp32 max)
NPT = M // PT                 # 8

_STATE = {}
LAST_RESULTS = None


def _make_nc(loop_reps=0, dve_chunks=(3, 4, 3), ft_bufs=4, psx_bufs=3):
    """Build the SPMD program.

    loop_reps>0 wraps the compute in a hardware For_i loop (benchmark-only).
    dve_chunks: chunks built on VectorE per (n,pt), alternating by pt parity;
    the rest go to GpSimdE.
    """
    import contextlib

    import concourse.bass as bass
    import concourse.mybir as mybir
    from concourse.tile import TileContext

    f32 = mybir.dt.float32
    f32r = mybir.dt.float32r

    nc = bass.Bass()
    x_d = nc.dram_tensor("x", [NLOC, C, M], f32, kind="ExternalInput")
    coef_d = nc.dram_tensor("coef", [NCHUNK, 128, M], f32, kind="ExternalInput")
    w_d = nc.dram_tensor("w", [NCHUNK, 128, COUT], f32, kind="ExternalInput")
    out_d = nc.dram_tensor("out", [NLOC, COUT, M], f32, kind="ExternalOutput")

    with TileContext(nc) as tc:
        with (
            tc.tile_pool(name="const", bufs=1) as cpool,
            tc.tile_pool(name="wstage", bufs=1) as wpool,
            tc.tile_pool(name="ft", bufs=ft_bufs) as ftpool,
            tc.tile_pool(name="osb", bufs=4) as opool,
            tc.tile_pool(name="ps", bufs=3, space="PSUM") as pspool,
            tc.tile_pool(name="psx", bufs=psx_bufs, space="PSUM") as psxpool,
        ):
            # const loads: HBM-in is only x (2MB) + small coef (320KB) + w;
            # the x4 / x32 partition replication runs as SBUF->SBUF DMA
            # log-doubling, which rides the fabric alongside HBM loads.
            xrep = [cpool.tile([128, M], f32, tag=f"xrep{n}", name=f"xr{n}")
                    for n in range(NLOC)]
            # all coef chunks in one tile: columns [q*M + p]
            coef_sb = cpool.tile([128, NCHUNK * M], f32, tag="coef")

            # weights first: tiny, and every matmul waits on them
            w_f32 = wpool.tile([128, NCHUNK * COUT], f32, tag="wf32")
            w_sb = cpool.tile([128, NCHUNK * COUT], f32r, tag="w")
            nc.sync.dma_start(
                w_f32[:, :].rearrange("p (q o) -> p q o", q=NCHUNK),
                w_d[:].rearrange("q p o -> p q o"),
            )
            nc.vector.tensor_copy(w_sb[:, :], w_f32[:, :])

            # xrep[0] + coef column-block 0 first, rest interleaved, so the
            # first compute tiles unblock while later loads stream in.
            CBW = M // NLOC
            for j in range(4):
                nc.sync.dma_start(xrep[0][32 * j:32 * (j + 1), :], x_d[0])
            for b in range(NLOC):
                for q in range(NCHUNK):
                    nc.sync.dma_start(
                        coef_sb[:, q * M + b * CBW: q * M + (b + 1) * CBW],
                        coef_d[q, :, b * CBW:(b + 1) * CBW],
                    )
                if b + 1 < NLOC:
                    n = b + 1
                    for j in range(4):
                        nc.sync.dma_start(xrep[n][32 * j:32 * (j + 1), :], x_d[n])

            loop_cm = tc.For_i(0, loop_reps, 1) if loop_reps else contextlib.nullcontext()
            with loop_cm:
                for n in range(NLOC):
                    for pt in range(NPT):
                        ndve = dve_chunks[pt % len(dve_chunks)]
                        ngp = NCHUNK - ndve
                        psl = slice(pt * PT, (pt + 1) * PT)

                        # stage x tile into PSUM (ScalarE) for VectorE's TT
                        psx = psxpool.tile([128, PT], f32)
                        nc.scalar.copy(psx[:, :], xrep[n][:, psl])

                        ft = ftpool.tile([128, NCHUNK * PT], f32r)
                        # VectorE: chunks [0, ndve) — x from PSUM, coef via
                        # dedicated SBUF read port; shared pair stays free.
                        coef3d = coef_sb[:, :].rearrange("r (q m) -> r q m", q=NCHUNK)
                        nc.vector.tensor_mul(
                            ft[:, : ndve * PT].rearrange(
                                "r (q p) -> r q p", q=ndve
                            ),
                            psx[:, :].unsqueeze(1).broadcast_to(
                                [128, ndve, PT]
                            ),
                            coef3d[:, :ndve, psl],
                        )
                        # GpSimdE: chunks [ndve, NCHUNK) from SBUF via the
                        # shared port pair.
                        nc.gpsimd.tensor_mul(
                            ft[:, ndve * PT:].rearrange(
                                "r (q p) -> r q p", q=ngp
                            ),
                            xrep[n][:, psl].unsqueeze(1).broadcast_to(
                                [128, ngp, PT]
                            ),
                            coef3d[:, ndve:, psl],
                        )

                        ps = pspool.tile([COUT, PT], f32)
                        for q in range(NCHUNK):
                            nc.tensor.matmul(
                                ps[:, :],
                                w_sb[:, q * COUT:(q + 1) * COUT],
                                ft[:, q * PT:(q + 1) * PT],
                                start=(q == 0),
                                stop=(q == NCHUNK - 1),
                            )
                        osb = opool.tile([COUT, PT], f32)
                        nc.scalar.copy(osb[:, :], ps[:, :])
                        nc.sync.dma_start(out_d[n, :, psl], osb[:, :])

    # split multi-sem waits into EventSemaphore insts (TRN2: 1 wait/inst)
    import bass_rust
    bass_rust.generate_event_semaphores(nc)
    return nc


def _prep_host(x, real_weights, imag_weights, diag_L):
    x = np.ascontiguousarray(np.asarray(x, dtype=np.float32).reshape(N, C, M))
    wr = np.asarray(real_weights, dtype=np.float32)
    wi = np.asarray(imag_weights, dtype=np.float32)
    d = np.asarray(diag_L, dtype=np.float64)

    s = (d - 1j) ** 2 / (d * d + 1.0)
    coefs = np.empty((NGRP, M), dtype=np.float32)
    ck = np.ones(M, dtype=np.complex128)
    for k in range(KTOT):
        coefs[k] = ck.real
        coefs[KTOT + k] = ck.imag
        ck = ck * s

    coef_pad = np.zeros((NCHUNK * 128, M), dtype=np.float32)
    coef_pad[:NGRP * C] = np.repeat(coefs, C, axis=0)
    w_pad = np.zeros((NCHUNK * 128, COUT), dtype=np.float32)
    w_pad[: KTOT * C] = 2.0 * wr
    w_pad[KTOT * C: NGRP * C] = 2.0 * wi
    return (
        x,
        np.ascontiguousarray(coef_pad.reshape(NCHUNK, 128, M)),
        np.ascontiguousarray(w_pad.reshape(NCHUNK, 128, COUT)),
    )


def _in_map(args, i):
    x3, coef, w = args
    return {"x": x3[NLOC * i: NLOC * (i + 1)], "coef": coef, "w": w}


def kernel(x, real_weights, imag_weights, diag_L, vals, rows, cols):
    global LAST_RESULTS
    from concourse.bass_utils import run_bass_kernel_spmd

    args = _prep_host(x, real_weights, imag_weights, diag_L)

    if "nc" not in _STATE:
        _STATE["nc"] = _make_nc()
    nc = _STATE["nc"]

    in_maps = [_in_map(args, i) for i in range(NCORES)]
    res = run_bass_kernel_spmd(nc, in_maps, list(range(NCORES)))
    LAST_RESULTS = res

    out = np.empty((N, M, COUT), dtype=np.float32)
    for i in range(NCORES):
        o = res.results[i]["out"]            # (NLOC, COUT, M)
        for j in range(NLOC):
            out[NLOC * i + j] = o[j].T
    return out.reshape(N, MSIDE, MSIDE, COUT)



# revision 9
# speedup vs baseline: 3.1939x; 3.1939x over previous
"""Trainium2 Bass kernel for nn_CayleyFilter (gnn_message_passing).

Math: the reference's Jacobi step degenerates — its SpMM terms cancel
algebraically:
    tr = (offr + diag*zr) + zi - offr == diag*zr + zi   (+- fp rounding noise)
    ti = (offi + diag*zi) - zr - offi == diag*zi - zr
so each Cayley iteration is an elementwise multiply by the per-node
unit-modulus complex scalar s_p = (d_p - i)^2 / (d_p^2 + 1).  Hence
    z_k = s^k (x)   and the whole module collapses to one GEMM:
    out[(n,p), o] = sum_{g,c} coef_g[p] * x[n,c,p] * W2[(g,c), o]
with coef_g in {Re(s^k), Im(s^k)} and W2 = 2*[Wr; Wi].

v2 layout (vs the fp32 batch-parallel v1 at 62.6us/iter):
  - shard over p (M) across the 8 cores: each core owns a 512-column
    slice of the graph for ALL 32 batches.  coef shrinks 8x per core.
  - fp16 operands end-to-end: DVE tensor_tensor runs in 2x_1P mode
    (the fp32 build ran 1x and dominated), DMA volume halves, the
    fp16 matmul is full-rate, PSUM accumulation stays fp32.
  - contraction = exactly the 16 nontrivial groups (k=1..8, re/im) =
    4 chunks of 128 rows; the k=0-real group (coef==1) is a direct
    K=32 matmul on x (no elementwise work, no copy); k=0-imag (==0)
    is dropped entirely.
  - matmuls col-tiled in pairs via tile_position (0,0)/(0,64): two
    batch-items share the 128 PSUM partitions -> ~2x TensorE.
  - fast path when diag_L is constant (e.g. all-ones): coefs are
    p-independent and fold into W on the host; the module collapses
    to a single K=32 GEMM, row+col-tiled 8 ways.
"""

import os

import numpy as np

N, C, M, MSIDE, COUT, ORDER = 32, 32, 4096, 64, 64, 8
NCORES = 8
KTOT = ORDER + 1              # 9
NGRP = 2 * ORDER              # 16 nontrivial coefficient groups
NCHUNK = 4                    # contraction chunks of 128 rows
PT = 512                      # p-columns per core (M / NCORES)
NPAIR = N // 2                # 16 col-tiled item pairs

_STATE = {}
LAST_RESULTS = None


def _make_nc(loop_reps=0, dve_chunks=(3, 3, 3, 4)):
    """General-path SPMD program (one 512-wide p-slice per core).

    loop_reps>0 wraps the compute in a hardware For_i loop (bench-only).
    dve_chunks[i % len]: chunks built on VectorE for item i; the rest
    of the 4 chunks go to GpSimdE.
    """
    import contextlib

    import concourse.bass as bass
    import concourse.mybir as mybir
    from concourse.tile import TileContext

    f32 = mybir.dt.float32
    f16 = mybir.dt.float16

    nc = bass.Bass()
    x_d = nc.dram_tensor("x", [128, N, PT], f16, kind="ExternalInput")
    coef_d = nc.dram_tensor("coef", [128, NCHUNK, PT], f16, kind="ExternalInput")
    w_d = nc.dram_tensor("w", [128, NCHUNK, COUT], f16, kind="ExternalInput")
    wt_d = nc.dram_tensor("wt", [64, COUT], f16, kind="ExternalInput")
    out_d = nc.dram_tensor("out", [NPAIR, 128, PT], f16, kind="ExternalOutput")

    with TileContext(nc) as tc:
        with (
            tc.tile_pool(name="const", bufs=1) as cpool,
            tc.tile_pool(name="ft", bufs=4) as ftpool,
            tc.tile_pool(name="osb", bufs=4) as opool,
            tc.tile_pool(name="ps", bufs=4, space="PSUM") as pspool,
        ):
            # weights + coef first: small, one-time setup constants
            w_sb = cpool.tile([128, NCHUNK * COUT], f16, tag="w")
            wt_sb = cpool.tile([64, COUT], f16, tag="wt")
            coef_sb = cpool.tile([128, NCHUNK * PT], f16, tag="coef")

            nc.sync.dma_start(
                w_sb[:, :].rearrange("p (q o) -> p q o", q=NCHUNK), w_d[:]
            )
            nc.sync.dma_start(wt_sb[:, :], wt_d[:])
            nc.sync.dma_start(
                coef_sb[:, :].rearrange("p (q m) -> p q m", q=NCHUNK), coef_d[:]
            )
            # x streamed in 4-item blocks (per-block tiles so the bench
            # loop's reloads pipeline at block granularity)
            XB = 4
            xb = [cpool.tile([128, XB * PT], f16, tag=f"xb{b}", name=f"xb{b}")
                  for b in range(N // XB)]

            coef3d = coef_sb[:, :].rearrange("r (q m) -> r q m", q=NCHUNK)

            loop_cm = tc.For_i(0, loop_reps, 1) if loop_reps else contextlib.nullcontext()
            with loop_cm:
                for b in range(N // XB):
                    nc.sync.dma_start(
                        xb[b][:, :].rearrange("p (n m) -> p n m", n=XB),
                        x_d[:, b * XB:(b + 1) * XB, :],
                    )
                for pr in range(NPAIR):
                    items = (2 * pr, 2 * pr + 1)
                    fts = []
                    for j, it in enumerate(items):
                        ndve = dve_chunks[it % len(dve_chunks)]
                        ngp = NCHUNK - ndve
                        xs = xb[it // XB][:, (it % XB) * PT:(it % XB + 1) * PT]

                        ft = ftpool.tile([128, NCHUNK * PT], f16, name=f"ft{j}")
                        nc.vector.tensor_mul(
                            ft[:, : ndve * PT].rearrange(
                                "r (q p) -> r q p", q=ndve
                            ),
                            xs.unsqueeze(1).broadcast_to([128, ndve, PT]),
                            coef3d[:, :ndve, :],
                        )
                        if ngp:
                            nc.gpsimd.tensor_mul(
                                ft[:, ndve * PT:].rearrange(
                                    "r (q p) -> r q p", q=ngp
                                ),
                                xs.unsqueeze(1).broadcast_to([128, ngp, PT]),
                                coef3d[:, ndve:, :],
                            )
                        fts.append(ft)

                    ps = pspool.tile([128, PT], f32)
                    halves = (ps[0:64, :], ps[64:128, :])
                    for q in range(NCHUNK):
                        for j in range(2):
                            nc.tensor.matmul(
                                halves[j],
                                w_sb[:, q * COUT:(q + 1) * COUT],
                                fts[j][:, q * PT:(q + 1) * PT],
                                start=(q == 0),
                                stop=False,
                                tile_position=(0, 64 * j),
                                skip_group_check=True,
                            )
                    # k=0-real tail: K=32 matmul straight on x, row-tiled
                    # so the two halves' tails run concurrently
                    for j, it in enumerate(items):
                        nc.tensor.matmul(
                            halves[j],
                            wt_sb[32 * j:32 * (j + 1), :],
                            xb[it // XB][
                                32 * j:32 * (j + 1),
                                (it % XB) * PT:(it % XB + 1) * PT,
                            ],
                            start=False,
                            stop=True,
                            tile_position=(32 * j, 64 * j),
                            skip_group_check=True,
                        )
                    osb = opool.tile([128, PT], f16)
                    nc.scalar.copy(osb[:, :], ps[:, :])
                    nc.sync.dma_start(out_d[pr], osb[:, :])

    import bass_rust
    bass_rust.generate_event_semaphores(nc)
    return nc


def _make_nc_const(loop_reps=0):
    """Fast path: diag_L constant => coefs fold into W; one K=32 GEMM.

    x packed [128 = 4 n-sub x 32 c, N/4 n-grp, PT]: the 4 n-subs are 4
    row-groups of the PE array, pairs of items are 2 col-groups -> 8
    concurrent K=32 matmuls.
    """
    import contextlib

    import concourse.bass as bass
    import concourse.mybir as mybir
    from concourse.tile import TileContext

    f32 = mybir.dt.float32
    f16 = mybir.dt.float16
    NG = N // 4               # 8 n-groups of 4 items

    nc = bass.Bass()
    x_d = nc.dram_tensor("x", [128, NG, PT], f16, kind="ExternalInput")
    w_d = nc.dram_tensor("w", [128, COUT], f16, kind="ExternalInput")
    out_d = nc.dram_tensor("out", [NPAIR, 128, PT], f16, kind="ExternalOutput")

    with TileContext(nc) as tc:
        with (
            tc.tile_pool(name="const", bufs=1) as cpool,
            tc.tile_pool(name="osb", bufs=4) as opool,
            tc.tile_pool(name="ps", bufs=4, space="PSUM") as pspool,
        ):
            # w: Wfold replicated on all 4 row-groups (partitions 4x32)
            w_sb = cpool.tile([128, COUT], f16, tag="w")
            xs = [cpool.tile([128, PT], f16, tag=f"x{b}", name=f"x{b}") for b in range(NG)]
            nc.sync.dma_start(w_sb[:, :], w_d[:])

            loop_cm = tc.For_i(0, loop_reps, 1) if loop_reps else contextlib.nullcontext()
            with loop_cm:
                for b in range(NG):
                    nc.sync.dma_start(xs[b][:, :], x_d[:, b, :])
                for b in range(NG):  # items 4b..4b+3
                    pss = [pspool.tile([128, PT], f32, name=f"ps{k}") for k in range(2)]
                    for sub in range(4):     # item 4b+sub; pair k=sub//2
                        ps = pss[sub // 2]
                        nc.tensor.matmul(
                            ps[64 * (sub % 2):64 * (sub % 2 + 1), :],
                            w_sb[32 * sub:32 * (sub + 1), :],
                            xs[b][32 * sub:32 * (sub + 1), :],
                            start=True,
                            stop=True,
                            tile_position=(32 * sub, 64 * (sub % 2)),
                            skip_group_check=True,
                        )
                    for k in range(2):
                        osb = opool.tile([128, PT], f16, name=f"o{k}")
                        nc.scalar.copy(osb[:, :], pss[k][:, :])
                        nc.sync.dma_start(out_d[2 * b + k], osb[:, :])

    import bass_rust
    bass_rust.generate_event_semaphores(nc)
    return nc


def _coefs(diag_L):
    """[16, M] nontrivial coefficient rows: Re(s^k), Im(s^k), k=1..8."""
    d = np.asarray(diag_L, dtype=np.float64)
    s = (d - 1j) ** 2 / (d * d + 1.0)
    out = np.empty((NGRP, d.shape[0]), dtype=np.float64)
    ck = s.copy()
    for k in range(ORDER):
        out[k] = ck.real
        out[ORDER + k] = ck.imag
        ck = ck * s
    return out


def _prep_host(x, real_weights, imag_weights, diag_L):
    x3 = np.asarray(x, dtype=np.float32).reshape(N, C, M)
    wr = np.asarray(real_weights, dtype=np.float64).reshape(KTOT, C, COUT)
    wi = np.asarray(imag_weights, dtype=np.float64).reshape(KTOT, C, COUT)
    coefs = _coefs(diag_L)

    # xr[g*32+c, n, p] = x[n, c, p]  (x4 over partition groups)
    xt = x3.transpose(1, 0, 2).astype(np.float16)           # [c, n, p]
    xr = np.broadcast_to(xt[None], (4, C, N, M)).reshape(128, N, M)

    # coef[gs*32+c, q, p] = coefs[4q+gs, p]
    cf = np.repeat(coefs.reshape(NCHUNK, 4, 1, M), C, axis=2)   # [q, gs, c, p]
    cf = np.ascontiguousarray(
        cf.transpose(1, 2, 0, 3).reshape(128, NCHUNK, M)
    ).astype(np.float16)

    # w[gs*32+c, q, o] = 2 * W_{4q+gs}[c, o]; groups = [re k=1..8, im k=1..8]
    wall = 2.0 * np.concatenate([wr[1:], wi[1:]], axis=0)       # [16, c, o]
    w = np.ascontiguousarray(
        wall.reshape(NCHUNK, 4, C, COUT).transpose(1, 2, 0, 3).reshape(
            128, NCHUNK, COUT
        )
    ).astype(np.float16)

    # tail: k=0-real (coef==1) on two row-group copies
    wt = np.ascontiguousarray(
        np.broadcast_to(2.0 * wr[0], (2, C, COUT)).reshape(64, COUT)
    ).astype(np.float16)
    return xr, cf, w, wt


def _prep_host_const(x, real_weights, imag_weights, diag_L):
    x3 = np.asarray(x, dtype=np.float32).reshape(N, C, M)
    wr = np.asarray(real_weights, dtype=np.float64).reshape(KTOT, C, COUT)
    wi = np.asarray(imag_weights, dtype=np.float64).reshape(KTOT, C, COUT)
    coefs = _coefs(diag_L[:1])                                  # [16, 1]
    # Wfold[c, o] = 2*Wr0 + sum_g coef_g * W_g
    wfold = 2.0 * wr[0] + np.tensordot(
        coefs[:, 0], 2.0 * np.concatenate([wr[1:], wi[1:]], axis=0), axes=(0, 0)
    )
    # x packed [sub*32+c, ngrp, p], n = ngrp*4 + sub
    xt = x3.reshape(N // 4, 4, C, M).transpose(1, 2, 0, 3)      # [sub, c, ng, p]
    xp = np.ascontiguousarray(xt.reshape(128, N // 4, M)).astype(np.float16)
    wf = np.ascontiguousarray(
        np.broadcast_to(wfold, (4, C, COUT)).reshape(128, COUT)
    ).astype(np.float16)
    return xp, wf


def _in_map_general(args, i):
    xr, cf, w, wt = args
    sl = slice(PT * i, PT * (i + 1))
    return {
        "x": np.ascontiguousarray(xr[:, :, sl]),
        "coef": np.ascontiguousarray(cf[:, :, sl]),
        "w": w,
        "wt": wt,
    }


def _in_map_const(args, i):
    xp, wf = args
    sl = slice(PT * i, PT * (i + 1))
    return {"x": np.ascontiguousarray(xp[:, :, sl]), "w": wf}


def _assemble(results):
    """res['out'][pair, part, p]: n = 2*pair + part//64, o = part%64."""
    out = np.empty((N, M, COUT), dtype=np.float32)
    for i in range(NCORES):
        o = np.asarray(results[i]["out"], dtype=np.float32)     # [16, 128, 512]
        o = o.reshape(NPAIR, 2, COUT, PT).transpose(0, 1, 3, 2) # [pair, b, p, o]
        out[:, PT * i:PT * (i + 1), :] = o.reshape(N, PT, COUT)
    return out.reshape(N, MSIDE, MSIDE, COUT)


def kernel(x, real_weights, imag_weights, diag_L, vals, rows, cols):
    global LAST_RESULTS
    from concourse.bass_utils import run_bass_kernel_spmd

    diag = np.asarray(diag_L, dtype=np.float32)
    const_diag = bool(np.all(diag == diag[0]))
    if os.environ.get("CAYLEY_FORCE_GENERAL"):
        const_diag = False

    if const_diag:
        args = _prep_host_const(x, real_weights, imag_weights, diag)
        if "nc_const" not in _STATE:
            _STATE["nc_const"] = _make_nc_const()
        nc = _STATE["nc_const"]
        in_maps = [_in_map_const(args, i) for i in range(NCORES)]
    else:
        args = _prep_host(x, real_weights, imag_weights, diag)
        if "nc" not in _STATE:
            _STATE["nc"] = _make_nc()
        nc = _STATE["nc"]
        in_maps = [_in_map_general(args, i) for i in range(NCORES)]

    res = run_bass_kernel_spmd(nc, in_maps, list(range(NCORES)))
    LAST_RESULTS = res
    return _assemble(res.results)


# revision 12
# speedup vs baseline: 3.9533x; 1.2378x over previous
"""Trainium2 Bass kernel for nn_CayleyFilter (gnn_message_passing).

Math: the reference's Jacobi step degenerates — its SpMM terms cancel
algebraically:
    tr = (offr + diag*zr) + zi - offr == diag*zr + zi   (+- fp rounding noise)
    ti = (offi + diag*zi) - zr - offi == diag*zi - zr
so each Cayley iteration is an elementwise multiply by the per-node
unit-modulus complex scalar s_p = (d_p - i)^2 / (d_p^2 + 1).  Hence
    z_k = s^k (x)   and the whole module collapses to one GEMM:
    out[(n,p), o] = sum_{g,c} coef_g[p] * x[n,c,p] * W2[(g,c), o]
with coef_g in {Re(s^k), Im(s^k)} and W2 = 2*[Wr; Wi].

v2 layout (vs the fp32 batch-parallel v1 at 62.6us/iter):
  - shard over p (M) across the 8 cores: each core owns a 512-column
    slice of the graph for ALL 32 batches.  coef shrinks 8x per core.
  - fp16 operands end-to-end: DVE tensor_tensor runs in 2x_1P mode
    (the fp32 build ran 1x and dominated), DMA volume halves, the
    fp16 matmul is full-rate, PSUM accumulation stays fp32.
  - contraction = exactly the 16 nontrivial groups (k=1..8, re/im) =
    4 chunks of 128 rows; the k=0-real group (coef==1) is a direct
    K=32 matmul on x (no elementwise work, no copy); k=0-imag (==0)
    is dropped entirely.
  - matmuls col-tiled in pairs via tile_position (0,0)/(0,64): two
    batch-items share the 128 PSUM partitions -> ~2x TensorE.
  - fast path when diag_L is constant (e.g. all-ones): coefs are
    p-independent and fold into W on the host; the module collapses
    to a single K=32 GEMM, row+col-tiled 8 ways.
"""

import os

import numpy as np

N, C, M, MSIDE, COUT, ORDER = 32, 32, 4096, 64, 64, 8
NCORES = 8
KTOT = ORDER + 1              # 9
NGRP = 2 * ORDER              # 16 nontrivial coefficient groups
NCHUNK = 4                    # contraction chunks of 128 rows
PT = 512                      # p-columns per core (M / NCORES)
NPAIR = N // 2                # 16 col-tiled item pairs

_STATE = {}
LAST_RESULTS = None


def _make_nc(loop_reps=0, dve_chunks=(3, 3, 3, 4)):
    """General-path SPMD program (one 512-wide p-slice per core).

    loop_reps>0 wraps the compute in a hardware For_i loop (bench-only).
    dve_chunks[i % len]: chunks built on VectorE for item i; the rest
    of the 4 chunks go to GpSimdE.
    """
    import contextlib

    import concourse.bass as bass
    import concourse.mybir as mybir
    from concourse.tile import TileContext

    f32 = mybir.dt.float32
    f16 = mybir.dt.float16

    nc = bass.Bass()
    x_d = nc.dram_tensor("x", [128, N, PT], f16, kind="ExternalInput")
    coef_d = nc.dram_tensor("coef", [128, NCHUNK, PT], f16, kind="ExternalInput")
    w_d = nc.dram_tensor("w", [128, NCHUNK, COUT], f16, kind="ExternalInput")
    wt_d = nc.dram_tensor("wt", [64, COUT], f16, kind="ExternalInput")
    out_d = nc.dram_tensor("out", [NPAIR, 128, PT], f16, kind="ExternalOutput")

    with TileContext(nc) as tc:
        with (
            tc.tile_pool(name="const", bufs=1) as cpool,
            tc.tile_pool(name="ft", bufs=4) as ftpool,
            tc.tile_pool(name="osb", bufs=4) as opool,
            tc.tile_pool(name="ps", bufs=4, space="PSUM") as pspool,
        ):
            # weights + coef first: small, one-time setup constants
            w_sb = cpool.tile([128, NCHUNK * COUT], f16, tag="w")
            wt_sb = cpool.tile([64, COUT], f16, tag="wt")
            coef_sb = cpool.tile([128, NCHUNK * PT], f16, tag="coef")

            nc.sync.dma_start(
                w_sb[:, :].rearrange("p (q o) -> p q o", q=NCHUNK), w_d[:]
            )
            nc.sync.dma_start(wt_sb[:, :], wt_d[:])
            nc.sync.dma_start(
                coef_sb[:, :].rearrange("p (q m) -> p q m", q=NCHUNK), coef_d[:]
            )
            # x streamed in 4-item blocks (per-block tiles so the bench
            # loop's reloads pipeline at block granularity)
            XB = 4
            xb = [cpool.tile([128, XB * PT], f16, tag=f"xb{b}", name=f"xb{b}")
                  for b in range(N // XB)]

            coef3d = coef_sb[:, :].rearrange("r (q m) -> r q m", q=NCHUNK)

            loop_cm = tc.For_i(0, loop_reps, 1) if loop_reps else contextlib.nullcontext()
            with loop_cm:
                for b in range(N // XB):
                    nc.sync.dma_start(
                        xb[b][:, :].rearrange("p (n m) -> p n m", n=XB),
                        x_d[:, b * XB:(b + 1) * XB, :],
                    )
                for pr in range(NPAIR):
                    items = (2 * pr, 2 * pr + 1)
                    fts = []
                    for j, it in enumerate(items):
                        ndve = dve_chunks[it % len(dve_chunks)]
                        ngp = NCHUNK - ndve
                        xs = xb[it // XB][:, (it % XB) * PT:(it % XB + 1) * PT]

                        ft = ftpool.tile([128, NCHUNK * PT], f16, name=f"ft{j}")
                        nc.vector.tensor_mul(
                            ft[:, : ndve * PT].rearrange(
                                "r (q p) -> r q p", q=ndve
                            ),
                            xs.unsqueeze(1).broadcast_to([128, ndve, PT]),
                            coef3d[:, :ndve, :],
                        )
                        if ngp:
                            nc.gpsimd.tensor_mul(
                                ft[:, ndve * PT:].rearrange(
                                    "r (q p) -> r q p", q=ngp
                                ),
                                xs.unsqueeze(1).broadcast_to([128, ngp, PT]),
                                coef3d[:, ndve:, :],
                            )
                        fts.append(ft)

                    ps = pspool.tile([128, PT], f32)
                    halves = (ps[0:64, :], ps[64:128, :])
                    for q in range(NCHUNK):
                        for j in range(2):
                            nc.tensor.matmul(
                                halves[j],
                                w_sb[:, q * COUT:(q + 1) * COUT],
                                fts[j][:, q * PT:(q + 1) * PT],
                                start=(q == 0),
                                stop=False,
                                tile_position=(0, 64 * j),
                                skip_group_check=True,
                            )
                    # k=0-real tail: K=32 matmul straight on x, row-tiled
                    # so the two halves' tails run concurrently
                    for j, it in enumerate(items):
                        nc.tensor.matmul(
                            halves[j],
                            wt_sb[32 * j:32 * (j + 1), :],
                            xb[it // XB][
                                32 * j:32 * (j + 1),
                                (it % XB) * PT:(it % XB + 1) * PT,
                            ],
                            start=False,
                            stop=True,
                            tile_position=(32 * j, 64 * j),
                            skip_group_check=True,
                        )
                    osb = opool.tile([128, PT], f16)
                    nc.scalar.copy(osb[:, :], ps[:, :])
                    nc.sync.dma_start(out_d[pr], osb[:, :])

    import bass_rust
    bass_rust.generate_event_semaphores(nc)
    return nc


def _make_nc_const(loop_reps=0):
    """Fast path: diag_L constant => coefs fold into W; one K=32 GEMM.

    x packed [128 = 4 n-sub x 32 c, N/4 n-grp, PT]: the 4 n-subs are 4
    row-groups of the PE array, pairs of items are 2 col-groups -> 8
    concurrent K=32 matmuls.
    """
    import contextlib

    import concourse.bass as bass
    import concourse.mybir as mybir
    from concourse.tile import TileContext

    f32 = mybir.dt.float32
    f16 = mybir.dt.float16
    NG = N // 4               # 8 n-groups of 4 items

    nc = bass.Bass()
    x_d = nc.dram_tensor("x", [128, NG, PT], f16, kind="ExternalInput")
    w_d = nc.dram_tensor("w", [128, COUT], f16, kind="ExternalInput")
    out_d = nc.dram_tensor("out", [NG // 2, 128, 4 * PT], f16, kind="ExternalOutput")

    with TileContext(nc) as tc:
        with (
            tc.tile_pool(name="const", bufs=1) as cpool,
            tc.tile_pool(name="osb", bufs=3) as opool,
            tc.tile_pool(name="ps", bufs=4, space="PSUM") as pspool,
        ):
            # w: Wfold replicated on all 4 row-groups (partitions 4x32)
            w_sb = cpool.tile([128, COUT], f16, tag="w")
            # x in 2-n-group tiles: few DMAs (HWDGE descriptor gen is the
            # const-path bottleneck, not data) but early compute unblocking
            xs = [cpool.tile([128, 2 * PT], f16, tag=f"x{t}", name=f"x{t}")
                  for t in range(NG // 2)]
            nc.sync.dma_start(w_sb[:, :], w_d[:])

            loop_cm = tc.For_i(0, loop_reps, 1) if loop_reps else contextlib.nullcontext()
            with loop_cm:
                for t in range(NG // 2):
                    nc.sync.dma_start(
                        xs[t][:, :].rearrange("p (u m) -> p u m", u=2),
                        x_d[:, 2 * t:2 * t + 2, :],
                    )
                for t in range(NG // 2):     # 2 n-groups = items 8t..8t+7
                    osb = opool.tile([128, 4 * PT], f16)
                    for u in range(2):       # n-group b = 2t+u
                        pss = [pspool.tile([128, PT], f32, name=f"ps{k}")
                               for k in range(2)]
                        for sub in range(4):  # item 4b+sub; pair k=sub//2
                            ps = pss[sub // 2]
                            nc.tensor.matmul(
                                ps[64 * (sub % 2):64 * (sub % 2 + 1), :],
                                w_sb[32 * sub:32 * (sub + 1), :],
                                xs[t][32 * sub:32 * (sub + 1),
                                      u * PT:(u + 1) * PT],
                                start=True,
                                stop=True,
                                tile_position=(32 * sub, 64 * (sub % 2)),
                                skip_group_check=True,
                            )
                        # evac split: ScalarE + (idle) VectorE in parallel
                        o0 = osb[:, (2 * u) * PT:(2 * u + 1) * PT]
                        o1 = osb[:, (2 * u + 1) * PT:(2 * u + 2) * PT]
                        nc.scalar.copy(o0, pss[0][:, :])
                        nc.vector.tensor_copy(o1, pss[1][:, :])
                    nc.sync.dma_start(out_d[t], osb[:, :])

    import bass_rust
    bass_rust.generate_event_semaphores(nc)
    return nc


def _coefs(diag_L):
    """[16, M] nontrivial coefficient rows: Re(s^k), Im(s^k), k=1..8."""
    d = np.asarray(diag_L, dtype=np.float64)
    s = (d - 1j) ** 2 / (d * d + 1.0)
    out = np.empty((NGRP, d.shape[0]), dtype=np.float64)
    ck = s.copy()
    for k in range(ORDER):
        out[k] = ck.real
        out[ORDER + k] = ck.imag
        ck = ck * s
    return out


def _prep_host(x, real_weights, imag_weights, diag_L):
    x3 = np.asarray(x, dtype=np.float32).reshape(N, C, M)
    wr = np.asarray(real_weights, dtype=np.float64).reshape(KTOT, C, COUT)
    wi = np.asarray(imag_weights, dtype=np.float64).reshape(KTOT, C, COUT)
    coefs = _coefs(diag_L)

    # xr[g*32+c, n, p] = x[n, c, p]  (x4 over partition groups)
    xt = x3.transpose(1, 0, 2).astype(np.float16)           # [c, n, p]
    xr = np.broadcast_to(xt[None], (4, C, N, M)).reshape(128, N, M)

    # coef[gs*32+c, q, p] = coefs[4q+gs, p]
    cf = np.repeat(coefs.reshape(NCHUNK, 4, 1, M), C, axis=2)   # [q, gs, c, p]
    cf = np.ascontiguousarray(
        cf.transpose(1, 2, 0, 3).reshape(128, NCHUNK, M)
    ).astype(np.float16)

    # w[gs*32+c, q, o] = 2 * W_{4q+gs}[c, o]; groups = [re k=1..8, im k=1..8]
    wall = 2.0 * np.concatenate([wr[1:], wi[1:]], axis=0)       # [16, c, o]
    w = np.ascontiguousarray(
        wall.reshape(NCHUNK, 4, C, COUT).transpose(1, 2, 0, 3).reshape(
            128, NCHUNK, COUT
        )
    ).astype(np.float16)

    # tail: k=0-real (coef==1) on two row-group copies
    wt = np.ascontiguousarray(
        np.broadcast_to(2.0 * wr[0], (2, C, COUT)).reshape(64, COUT)
    ).astype(np.float16)
    return xr, cf, w, wt


def _prep_host_const(x, real_weights, imag_weights, diag_L):
    x3 = np.asarray(x, dtype=np.float32).reshape(N, C, M)
    wr = np.asarray(real_weights, dtype=np.float64).reshape(KTOT, C, COUT)
    wi = np.asarray(imag_weights, dtype=np.float64).reshape(KTOT, C, COUT)
    coefs = _coefs(diag_L[:1])                                  # [16, 1]
    # Wfold[c, o] = 2*Wr0 + sum_g coef_g * W_g
    wfold = 2.0 * wr[0] + np.tensordot(
        coefs[:, 0], 2.0 * np.concatenate([wr[1:], wi[1:]], axis=0), axes=(0, 0)
    )
    # x packed [sub*32+c, ngrp, p], n = ngrp*4 + sub
    xt = x3.reshape(N // 4, 4, C, M).transpose(1, 2, 0, 3)      # [sub, c, ng, p]
    xp = np.ascontiguousarray(xt.reshape(128, N // 4, M)).astype(np.float16)
    wf = np.ascontiguousarray(
        np.broadcast_to(wfold, (4, C, COUT)).reshape(128, COUT)
    ).astype(np.float16)
    return xp, wf


def _in_map_general(args, i):
    xr, cf, w, wt = args
    sl = slice(PT * i, PT * (i + 1))
    return {
        "x": np.ascontiguousarray(xr[:, :, sl]),
        "coef": np.ascontiguousarray(cf[:, :, sl]),
        "w": w,
        "wt": wt,
    }


def _in_map_const(args, i):
    xp, wf = args
    sl = slice(PT * i, PT * (i + 1))
    return {"x": np.ascontiguousarray(xp[:, :, sl]), "w": wf}


def _assemble(results, const):
    out = np.empty((N, M, COUT), dtype=np.float32)
    for i in range(NCORES):
        o = np.asarray(results[i]["out"], dtype=np.float32)
        if const:
            # out[t, (half, o), (u, k, p)]: n = 8t + 4u + 2k + half
            o = o.reshape(4, 2, COUT, 2, 2, PT).transpose(0, 3, 4, 1, 5, 2)
        else:
            # out[pair, (half, o), p]: n = 2*pair + half
            o = o.reshape(NPAIR, 2, COUT, PT).transpose(0, 1, 3, 2)
        out[:, PT * i:PT * (i + 1), :] = o.reshape(N, PT, COUT)
    return out.reshape(N, MSIDE, MSIDE, COUT)


def kernel(x, real_weights, imag_weights, diag_L, vals, rows, cols):
    global LAST_RESULTS
    from concourse.bass_utils import run_bass_kernel_spmd

    diag = np.asarray(diag_L, dtype=np.float32)
    const_diag = bool(np.all(diag == diag[0]))
    if os.environ.get("CAYLEY_FORCE_GENERAL"):
        const_diag = False

    if const_diag:
        args = _prep_host_const(x, real_weights, imag_weights, diag)
        if "nc_const" not in _STATE:
            _STATE["nc_const"] = _make_nc_const()
        nc = _STATE["nc_const"]
        in_maps = [_in_map_const(args, i) for i in range(NCORES)]
    else:
        args = _prep_host(x, real_weights, imag_weights, diag)
        if "nc" not in _STATE:
            _STATE["nc"] = _make_nc()
        nc = _STATE["nc"]
        in_maps = [_in_map_general(args, i) for i in range(NCORES)]

    res = run_bass_kernel_spmd(nc, in_maps, list(range(NCORES)))
    LAST_RESULTS = res
    return _assemble(res.results, const_diag)
